# revision 23
# baseline (speedup 1.0000x reference)
"""Trainium2 Bass kernel for a 2-layer GCN + edge score predictor (8-core SPMD).

Strategy (graph/data parallel, node-sharded):
  - Nodes are permuted into 8 cores x 49 blocks x 128 slots, balanced by
    in-degree. Each core owns the edges whose dst falls in its shard.
  - rsqrt(deg_out) folds into the gathered tables (host-prescaled x for L1,
    scale applied during the z copy for L2); rsqrt(deg_in) folds into the
    one-hot aggregation matrix M, which is shipped as host-packed one-hot
    BITS and expanded on-chip with two fast 16-bit DVE ops per block:
    AND(bits, 2^b masks) -> {0, 2^b}, then multiply by rs_in * 2^-b
    (exact powers of two).
  - Aggregation per dst-block: dma_gather of src rows (lo/hi int16 index
    halves) spread over all 4 SWDGE queues with a cost-balanced greedy
    schedule (queue 0 blocks the gpsimd engine, so it gets a reduced
    share), then a chain of PE matmuls vs M.
  - The edge predictor: score = y[esrc] + w[edst] + bp with (y|w) = x2@Wp.
    Each edge is scored on the core owning its endpoint node via one-hot
    selection matmuls against the SBUF-resident local yw tile. Selection
    one-hots are host-packed bits expanded on-chip (AND + is_equal, u16).
    Slots are grouped per node-tile with S_p=2560 so every 512-wide psum
    window maps to exactly one tile; the predictor is interleaved into the
    L2 block loop (tile t ready after block t). The two halves are summed
    on the host after unpermuting.
"""

import numpy as np

N = 50000
E = 800000
NC = 8
B = 49
BS = 128
SHARD = B * BS            # 6272
NTOT = NC * SHARD         # 50176
HALF = 32768              # lo/hi split of table rows for int16 gather indices
IN_D = 128
HID = 256
OUT_D = 128
NCLS = 16
S_P = 2560                # predictor slots per node-tile (5 windows)
WIN = 512                 # predictor psum window
QUAD = 2048               # predictor sel-build granularity (16 bits x 128 words)
SLOTS = B * S_P           # 125440
NWIN = SLOTS // WIN       # 245
NQUAD = (SLOTS + QUAD - 1) // QUAD   # 62
NWORD = NQUAD * 128       # bm16 words per partition


def _wrap16(idx_list, n_slots):
    a = np.zeros((16, n_slots // 16), np.int16)
    i = np.arange(n_slots)
    a[i % 16, i // 16] = idx_list
    return a


def _preprocess(input_features, src, dst, esrc, edst, W1, b1, W2, b2, Wp, bp):
    import ml_dtypes
    bf = ml_dtypes.bfloat16

    src = np.asarray(src)
    dst = np.asarray(dst)
    esrc = np.asarray(esrc)
    edst = np.asarray(edst)
    x = np.asarray(input_features, np.float32)

    deg_out = np.bincount(src, minlength=N).astype(np.float64)
    deg_in = np.bincount(dst, minlength=N).astype(np.float64)
    rs_out = (1.0 / np.sqrt(np.clip(deg_out, 1.0, None))).astype(np.float32)
    rs_in = (1.0 / np.sqrt(np.clip(deg_in, 1.0, None))).astype(np.float32)

    # node -> global slot permutation, in-degree balanced over the 392 blocks
    order = np.argsort(-deg_in, kind="stable")
    NBUCK = NC * B
    i = np.arange(N)
    bucket = i % NBUCK
    slot = i // NBUCK
    core = bucket % NC
    block = bucket // NC
    g = core * SHARD + block * BS + slot
    perm = np.empty(N, np.int64)
    perm[order] = g
    inv = np.full(NTOT, -1, np.int64)
    inv[perm] = np.arange(N)

    # ---- L1/L2 edge grouping by (dst core, dst block, src half) ----
    pd = perm[dst]
    ps = perm[src]
    e_core = pd // SHARD
    e_block = (pd % SHARD) // BS
    e_dslot = pd % BS
    e_hi = (ps >= HALF).astype(np.int64)

    key = (e_core * B + e_block) * 2 + e_hi
    sort_idx = np.argsort(key, kind="stable")
    counts = np.bincount(key, minlength=NC * B * 2).reshape(NC, B, 2)
    S_lo = int(np.ceil(counts[:, :, 0].max() / BS) * BS)
    S_hi = int(np.ceil(counts[:, :, 1].max() / BS) * BS)
    SBLK = S_lo + S_hi
    TOT = B * SBLK
    NT = SBLK // 128

    gidx = np.zeros((NC, TOT), np.int64)
    dloc = np.full((NC, TOT), -1, np.int64)

    ec = e_core[sort_idx]
    eb = e_block[sort_idx]
    eh = e_hi[sort_idx]
    gkey = (ec * B + eb) * 2 + eh
    grp_start = np.zeros(NC * B * 2 + 1, np.int64)
    np.cumsum(counts.reshape(-1), out=grp_start[1:])
    pos_in_grp = np.arange(E) - grp_start[gkey]
    slots = eb * SBLK + eh * S_lo + pos_in_grp
    gidx[ec, slots] = ps[sort_idx] - eh * HALF
    dloc[ec, slots] = e_dslot[sort_idx]

    idx16 = np.zeros((NC, 128, TOT // 16), np.int16)
    # one-hot bits of M: for slot-group (blk, t) partition p with dst j:
    # word (blk*NT + t)*8 + j%8 gets bit j//8
    bmM = np.zeros((NC, 128, (TOT // 128) * 8), np.uint16)
    iw = np.arange(SBLK)
    for c in range(NC):
        col = 0
        for b in range(B):
            for gi, S_g in enumerate((S_lo, S_hi)):
                s0 = b * SBLK + gi * S_lo
                idx16[c, :, col:col + S_g // 16] = np.tile(
                    _wrap16(gidx[c, s0:s0 + S_g], S_g), (8, 1))
                col += S_g // 16
        d = dloc[c]                      # [TOT]
        valid = d >= 0
        t_all = np.arange(TOT) // 128
        p_all = np.arange(TOT) % 128
        wcol = t_all * 8 + (d % 8)
        bits = (1 << (d // 8)).astype(np.uint16)
        np.bitwise_or.at(bmM[c], (p_all[valid], wcol[valid]), bits[valid])

    mskM = np.zeros((128, 16, 8), np.uint16)
    mskM[:, np.arange(16), :] = (1 << np.arange(16, dtype=np.uint32)
                                 ).astype(np.uint16)[None, :, None]

    # ---- per-core shards (permuted node order) ----
    x16 = np.zeros((NC, SHARD, IN_D), bf)
    rsin2 = np.zeros((NC, 128, SHARD), bf)   # rs_in * 2^-(j//8), replicated
    rsout_blk = np.zeros((NC, 128, B), np.float32)
    j_in_blk = np.arange(SHARD) % 128
    pow2 = (2.0 ** -(j_in_blk // 8)).astype(np.float32)
    for c in range(NC):
        nodes = inv[c * SHARD:(c + 1) * SHARD]
        m = nodes >= 0
        x16[c, m] = (x[nodes[m]] * rs_out[nodes[m], None]).astype(bf)
        ri = np.zeros(SHARD, np.float32)
        ri[m] = rs_in[nodes[m]]
        rsin2[c] = np.tile((ri * pow2).astype(bf), (128, 1))
        ro = np.zeros(SHARD, np.float32)
        ro[m] = rs_out[nodes[m]]
        rsout_blk[c] = ro.reshape(B, BS).T

    # ---- predictor: per-endpoint slots grouped by node tile ----
    tau_s = perm[esrc]
    tau_d = perm[edst]

    def pass_slots(tau):
        c_e = tau // SHARD
        t_e = (tau % SHARD) // BS
        loc = tau % BS
        k = c_e * B + t_e
        so = np.argsort(k, kind="stable")
        cnt = np.bincount(k, minlength=NC * B)
        assert cnt.max() <= S_P, f"predictor tile overflow: {cnt.max()}"
        gs = np.zeros(NC * B + 1, np.int64)
        np.cumsum(cnt, out=gs[1:])
        pos = np.empty(E, np.int64)
        pos[so] = np.arange(E) - gs[k[so]]
        slot = t_e * S_P + pos
        # bits: slot s -> word (s//2048)*128 + s%128, bit (s%2048)//128
        bm = np.zeros((NC, 128, NWORD), np.uint16)
        wcol = (slot // QUAD) * 128 + slot % 128
        bits = (1 << ((slot % QUAD) // 128)).astype(np.uint16)
        np.bitwise_or.at(bm, (c_e, loc, wcol), bits)
        return c_e, slot, bm

    cA, slotA, bmA = pass_slots(tau_s)
    cB, slotB, bmB = pass_slots(tau_d)

    mskP = np.zeros((128, 16, 128), np.uint16)
    mskP[:, np.arange(16), :] = (1 << np.arange(16, dtype=np.uint32)
                                 ).astype(np.uint16)[None, :, None]

    shared = dict(
        mskM=mskM.reshape(128, 128),
        mskP=mskP.reshape(128, 16 * 128),
        W1=np.asarray(W1, np.float32).astype(bf),                     # [128, 256]
        b1=np.asarray(b1, np.float32).reshape(2, 128).T.copy(),       # [128, 2]
        W2=np.concatenate([np.asarray(W2[:128], np.float32),
                           np.asarray(W2[128:], np.float32)], 1).astype(bf),  # [128, 256]
        b2=np.asarray(b2, np.float32).reshape(128, 1),
        Wp=np.concatenate([np.asarray(Wp[:OUT_D], np.float32),
                           np.asarray(Wp[OUT_D:], np.float32)], 1).astype(bf),  # [128, 32]
    )
    per_core = dict(x16=x16, idx16=idx16, bmM=bmM,
                    rsin2=rsin2, rsout=rsout_blk, bmA=bmA, bmB=bmB)
    meta = dict(S_lo=S_lo, S_hi=S_hi, SBLK=SBLK, TOT=TOT)
    host = dict(cA=cA, slotA=slotA, cB=cB, slotB=slotB,
                bp=np.asarray(bp, np.float32))
    return meta, shared, per_core, host


def _gather_schedule(S_lo, S_hi):
    """Greedy cost-balanced queue assignment with a min-reuse spacing of 3
    so the engine never stalls on a still-busy queue pair; queue 0 carries
    a cost weight because its descriptor generation blocks the engine."""
    w = [1.45, 1.0, 1.0, 1.0]
    loads = [0.0, 0.0, 0.0, 0.0]
    recent = []
    sched = []
    for b in range(B):
        qs = []
        for cost in (S_lo, S_hi):
            cand = sorted(range(4), key=lambda q: loads[q] + w[q] * cost)
            q = next((c for c in cand if c not in recent), cand[0])
            loads[q] += w[q] * cost
            recent = (recent + [q])[-2:]
            qs.append(q)
        sched.append(tuple(qs))
    return sched


def _build_program(meta, stop_after=None):
    import concourse.bacc as bacc
    import concourse.mybir as mybir
    import concourse.tile as tile

    dt = mybir.dt
    S_lo, S_hi, SBLK, TOT = meta["S_lo"], meta["S_hi"], meta["SBLK"], meta["TOT"]
    NLO = S_lo // 128
    NHI = S_hi // 128
    NT = SBLK // 128
    sched = _gather_schedule(S_lo, S_hi)

    # predictor gates: quad q ready after block gate_q[q]; window w after
    # gate_w[w]
    gate_q = [min(B - 1, (QUAD * (q + 1) - 1) // S_P) for q in range(NQUAD)]
    gate_w = [max(w // 5, gate_q[w // 4]) for w in range(NWIN)]

    nc = bacc.Bacc("TRN2", target_bir_lowering=False, debug=False,
                   num_devices=NC, num_swdge_queues=4)

    def din(name, shape, dtype):
        return nc.dram_tensor(name, shape, dtype, kind="ExternalInput")

    t_x16 = din("x16", [SHARD, IN_D], dt.bfloat16)
    t_idx = din("idx16", [128, TOT // 16], dt.int16)
    t_bmM = din("bmM", [128, (TOT // 128) * 8], dt.uint16)
    t_mskM = din("mskM", [128, 128], dt.uint16)
    t_rsin2 = din("rsin2", [128, SHARD], dt.bfloat16)
    t_rsout = din("rsout", [128, B], dt.float32)
    t_bmA = din("bmA", [128, NWORD], dt.uint16)
    t_bmB = din("bmB", [128, NWORD], dt.uint16)
    t_mskP = din("mskP", [128, 16 * 128], dt.uint16)
    t_W1 = din("W1", [128, HID], dt.bfloat16)
    t_b1 = din("b1", [128, 2], dt.float32)
    t_W2 = din("W2", [128, HID], dt.bfloat16)
    t_b2 = din("b2", [128, 1], dt.float32)
    t_Wp = din("Wp", [128, 32], dt.bfloat16)
    t_syA = nc.dram_tensor("syA", [NWIN, 16, WIN], dt.float32,
                           kind="ExternalOutput")
    t_syB = nc.dram_tensor("syB", [NWIN, 16, WIN], dt.float32,
                           kind="ExternalOutput")

    x_bounce = nc.dram_tensor("x_bounce", [SHARD, IN_D], dt.bfloat16)
    x_table = nc.dram_tensor("x_table", [NTOT, IN_D], dt.bfloat16,
                             addr_space="Shared")
    z_bounce = nc.dram_tensor("z_bounce", [SHARD, OUT_D], dt.bfloat16)
    z_table = nc.dram_tensor("z_table", [NTOT, OUT_D], dt.bfloat16,
                             addr_space="Shared")
    rg = [list(range(NC))]

    with tile.TileContext(nc) as tc:
        with tc.tile_pool(name="const", bufs=1) as cpool:
            nc.sync.dma_start(out=x_bounce.ap(), in_=t_x16.ap())
            nc.gpsimd.collective_compute(
                "AllGather", mybir.AluOpType.bypass, replica_groups=rg,
                ins=[x_bounce.ap().opt()], outs=[x_table.ap().opt()])

            mskM_sb = cpool.tile([128, 16, 8], dt.uint16)
            nc.sync.dma_start(
                out=mskM_sb[:],
                in_=t_mskM.ap().rearrange("p (b a) -> p b a", a=8))
            mskP_sb = cpool.tile([128, 16, 128], dt.uint16)
            nc.sync.dma_start(
                out=mskP_sb[:],
                in_=t_mskP.ap().rearrange("p (b a) -> p b a", a=128))
            W1_sb = cpool.tile([128, HID], dt.bfloat16)
            nc.sync.dma_start(out=W1_sb[:], in_=t_W1[:])
            b1_sb = cpool.tile([128, 2], dt.float32)
            nc.sync.dma_start(out=b1_sb[:], in_=t_b1[:])
            W2_sb = cpool.tile([128, HID], dt.bfloat16)
            nc.sync.dma_start(out=W2_sb[:], in_=t_W2[:])
            b2_sb = cpool.tile([128, 1], dt.float32)
            nc.sync.dma_start(out=b2_sb[:], in_=t_b2[:])
            Wp_sb = cpool.tile([128, 32], dt.bfloat16)
            nc.sync.dma_start(out=Wp_sb[:], in_=t_Wp[:])
            rsin2_sb = cpool.tile([128, SHARD], dt.bfloat16)
            nc.sync.dma_start(out=rsin2_sb[:], in_=t_rsin2[:])
            rsout_sb = cpool.tile([128, B], dt.float32)
            nc.sync.dma_start(out=rsout_sb[:], in_=t_rsout[:])
            bmA_sb = cpool.tile([128, NWORD], dt.uint16)
            nc.sync.dma_start(out=bmA_sb[:], in_=t_bmA[:])
            bmB_sb = cpool.tile([128, NWORD], dt.uint16)
            nc.sync.dma_start(out=bmB_sb[:], in_=t_bmB[:])
            Y_sb = cpool.tile([128, B, 32], dt.bfloat16)

            AND = mybir.AluOpType.bitwise_and
            EQ = mybir.AluOpType.is_equal
            MUL = mybir.AluOpType.mult

            with tc.tile_pool(name="l12", bufs=1) as lp, \
                 tc.tile_pool(name="gat", bufs=8) as gp, \
                 tc.tile_pool(name="mm", bufs=4) as mp, \
                 tc.tile_pool(name="psA", bufs=3, space="PSUM") as psA, \
                 tc.tile_pool(name="psB", bufs=1, space="PSUM") as psB, \
                 tc.tile_pool(name="psP", bufs=2, space="PSUM") as psP:
                idx_sb = lp.tile([128, TOT // 16], dt.int16)
                nc.sync.dma_start(out=idx_sb[:], in_=t_idx[:])
                bmM_sb = lp.tile([128, (TOT // 128) * 8], dt.uint16)
                nc.sync.dma_start(out=bmM_sb[:], in_=t_bmM[:])

                def issue_gathers(b, table):
                    """fire the two dma_gathers for block b, return tiles"""
                    ic = b * SBLK // 16
                    q_lo, q_hi = sched[b]
                    glo = gp.tile([128, NLO, 128], dt.bfloat16, tag="glo")
                    nc.gpsimd.dma_gather(
                        out_ap=glo[:], in_ap=table[:HALF, :],
                        idxs_ap=idx_sb[:, ic:ic + S_lo // 16],
                        num_idxs=S_lo, num_idxs_reg=S_lo, elem_size=128,
                        single_packet=False, queue_num=q_lo)
                    ghi = gp.tile([128, NHI, 128], dt.bfloat16, tag="ghi")
                    nc.gpsimd.dma_gather(
                        out_ap=ghi[:], in_ap=table[HALF:, :],
                        idxs_ap=idx_sb[:, ic + S_lo // 16:ic + SBLK // 16],
                        num_idxs=S_hi, num_idxs_reg=S_hi, elem_size=128,
                        single_packet=False, queue_num=q_hi)
                    return glo, ghi

                def agg_layer(b, tiles):
                    """one dst-block aggregation -> aggT PSUM tile [F, BS]"""
                    glo, ghi = tiles
                    # M = onehot(dloc) * rs_in[dst]: AND then scale by
                    # rs_in * 2^-b (exact)
                    tmpM = mp.tile([128, NT, 16, 8], dt.uint16, tag="tmpM")
                    nc.vector.tensor_tensor(
                        out=tmpM[:],
                        in0=bmM_sb[:, b * NT * 8:(b + 1) * NT * 8]
                        .rearrange("p (t a) -> p t a", a=8)[:, :, None, :]
                        .to_broadcast([128, NT, 16, 8]),
                        in1=mskM_sb[:, None, :, :].to_broadcast(
                            [128, NT, 16, 8]),
                        op=AND)
                    M = mp.tile([128, NT, 128], dt.bfloat16, tag="M")
                    nc.vector.tensor_tensor(
                        out=M[:].rearrange("p t (x y) -> p t x y", y=8),
                        in0=tmpM[:],
                        in1=rsin2_sb[:, None, b * BS:(b + 1) * BS]
                        .rearrange("p o (x y) -> p o x y", y=8)
                        .to_broadcast([128, NT, 16, 8]),
                        op=MUL)
                    aggT = psA.tile([128, BS], dt.float32, tag="aggT",
                                    space="PSUM")
                    for t in range(NT):
                        src_t = (glo[:, t, :] if t < NLO
                                 else ghi[:, t - NLO, :])
                        nc.tensor.matmul(aggT[:], lhsT=src_t, rhs=M[:, t, :],
                                         start=(t == 0), stop=(t == NT - 1))
                    return aggT

                # ---- phase 1: L1 + z ----
                PF = 2
                pend = {}
                for bb in range(B + PF):
                    if bb < B:
                        pend[bb] = issue_gathers(bb, x_table)
                    if bb < PF:
                        continue
                    b = bb - PF
                    aggT = agg_layer(b, pend.pop(b))
                    aggT_sb = mp.tile([128, BS], dt.bfloat16, tag="aggs")
                    nc.scalar.activation(
                        out=aggT_sb[:], in_=aggT[:],
                        func=mybir.ActivationFunctionType.Copy)
                    x1b = mp.tile([128, 2, 128], dt.bfloat16, tag="x1b")
                    for k in range(2):
                        o1 = psB.tile([128, BS], dt.float32, tag="o1",
                                      space="PSUM")
                        nc.tensor.matmul(
                            o1[:], lhsT=W1_sb[:, k * 128:(k + 1) * 128],
                            rhs=aggT_sb[:], start=True, stop=True)
                        nc.scalar.activation(
                            out=x1b[:, k, :], in_=o1[:],
                            func=mybir.ActivationFunctionType.Relu,
                            bias=b1_sb[:, k:k + 1], scale=1.0)
                    zp = psB.tile([128, OUT_D], dt.float32, tag="zp",
                                  space="PSUM")
                    for k in range(2):
                        nc.tensor.matmul(
                            zp[:], lhsT=x1b[:, k, :],
                            rhs=W2_sb[:, k * 128:(k + 1) * 128],
                            start=(k == 0), stop=(k == 1))
                    z_sb = mp.tile([128, OUT_D], dt.bfloat16, tag="zsb")
                    nc.scalar.activation(
                        out=z_sb[:], in_=zp[:],
                        func=mybir.ActivationFunctionType.Copy,
                        bias=0.0, scale=rsout_sb[:, b:b + 1])
                    nc.sync.dma_start(out=z_bounce[b * BS:(b + 1) * BS, :],
                                      in_=z_sb[:])
                nc.gpsimd.collective_compute(
                    "AllGather", mybir.AluOpType.bypass, replica_groups=rg,
                    ins=[z_bounce.ap().opt()], outs=[z_table.ap().opt()])

                # ---- phase 2: L2 + interleaved predictor ----
                with tc.tile_pool(name="pred", bufs=3) as pp:

                    def sel_quad(bm_sb, q, tag):
                        tmp = pp.tile([128, 16, 128], dt.uint16,
                                      tag=tag + "t")
                        nc.vector.tensor_tensor(
                            out=tmp[:],
                            in0=bm_sb[:, None, q * 128:(q + 1) * 128]
                            .to_broadcast([128, 16, 128]),
                            in1=mskP_sb[:], op=AND)
                        sel = pp.tile([128, 16, 128], dt.bfloat16, tag=tag)
                        nc.vector.tensor_tensor(
                            out=sel[:], in0=tmp[:], in1=mskP_sb[:], op=EQ)
                        return sel

                    sel_tiles = {}

                    def emit_windows(b):
                        for q in range(NQUAD):
                            if gate_q[q] == b:
                                sel_tiles[("A", q)] = sel_quad(bmA_sb, q, "sA")
                                sel_tiles[("B", q)] = sel_quad(bmB_sb, q, "sB")
                        for w in range(NWIN):
                            if gate_w[w] != b:
                                continue
                            t = w // 5
                            for ps, col0, t_out, on_scalar in (
                                    ("A", 0, t_syA, True),
                                    ("B", 16, t_syB, False)):
                                sel = sel_tiles[(ps, w // 4)]
                                sflat = sel[:].rearrange("p b a -> p (b a)")
                                c0 = (w % 4) * WIN
                                P = psP.tile([16, WIN], dt.float32, tag="P",
                                             space="PSUM")
                                nc.tensor.matmul(
                                    P[:], lhsT=Y_sb[:, t, col0:col0 + 16],
                                    rhs=sflat[:, c0:c0 + WIN],
                                    start=True, stop=True,
                                    skip_group_check=True)
                                P_sb = pp.tile([16, WIN], dt.float32,
                                               tag="Psb")
                                nc.scalar.activation(
                                    out=P_sb[:], in_=P[:],
                                    func=mybir.ActivationFunctionType.Copy)
                                nc.sync.dma_start(out=t_out.ap()[w],
                                                  in_=P_sb[:])

                    pend2 = {}
                    for bb in range(B + PF):
                        if bb < B:
                            pend2[bb] = issue_gathers(bb, z_table)
                        if bb < PF:
                            continue
                        b = bb - PF
                        aggT2 = agg_layer(b, pend2.pop(b))
                        x2b = mp.tile([128, BS], dt.bfloat16, tag="x2b")
                        nc.scalar.activation(
                            out=x2b[:], in_=aggT2[:],
                            func=mybir.ActivationFunctionType.Relu,
                            bias=b2_sb[:, 0:1], scale=1.0)
                        ywp = psB.tile([128, 32], dt.float32, tag="ywp",
                                       space="PSUM")
                        nc.tensor.matmul(ywp[:], lhsT=x2b[:], rhs=Wp_sb[:],
                                         start=True, stop=True)
                        nc.scalar.activation(
                            out=Y_sb[:, b, :], in_=ywp[:],
                            func=mybir.ActivationFunctionType.Copy)
                        emit_windows(b)

    nc.compile()
    return nc


def _make_in_maps(shared, per_core):
    in_maps = []
    for c in range(NC):
        m = dict(shared)
        for k in ("x16", "idx16", "bmM", "rsin2", "rsout", "bmA", "bmB"):
            m[k] = per_core[k][c]
        in_maps.append({k: np.ascontiguousarray(v) for k, v in m.items()})
    return in_maps


def _postprocess(results, host):
    def decode(name):
        sy = np.stack([np.asarray(results[c][name]) for c in range(NC)])
        # [NC, NWIN, 16, WIN] -> [NC, SLOTS, 16]
        return np.ascontiguousarray(sy.transpose(0, 1, 3, 2)).reshape(
            NC, SLOTS, NCLS)

    SA = decode("syA")
    SB = decode("syB")
    score = (SA[host["cA"], host["slotA"]] + SB[host["cB"], host["slotB"]]
             + host["bp"][None, :])
    return score.astype(np.float32)


def kernel(**inputs):
    from concourse.bass_utils import run_bass_kernel_spmd

    meta, shared, per_core, host = _preprocess(**inputs)
    nc = _build_program(meta)
    in_maps = _make_in_maps(shared, per_core)
    res = run_bass_kernel_spmd(nc, in_maps, list(range(NC)))
    return _postprocess(res.results, host)
